# revision 1
# baseline (speedup 1.0000x reference)
"""Trainium2 Bass kernel for a sparse-conv BasicBlock (gnn message passing).

Computation (see reference):
    out1 = relu(bn1(scatter_add(gather(x, idx_in) @ w1, idx_out)))
    out2 = bn2(scatter_add(gather(out1, idx_in) @ w2, idx_out))
    y    = relu(out2 + x)

Strategy (8 NeuronCores, one SPMD program):
  * Shard output voxels: core c owns rows [c*RPC, (c+1)*RPC).  Within a
    core, rows are re-assigned to 128-row blocks by a greedy balancer so
    per-(block, k) pair counts are nearly equal across cores (the single
    shared program uses max-over-cores slot counts; balancing cuts the
    padding).  All tensors live in "position space" (block*128 + rank);
    the host permutes inputs and inverse-permutes the output.
  * Host packs each core's (k, m) pairs by (block, k) into 128-slot
    chunks, k-major compact.  Dummy slots gather a zero row, keytag -1.
  * Per chunk: one indirect-DMA row-gather (128 rows, fp16) — the per-op
    SWDGE cost (~1.4us) dominates the kernel, so chunk count is minimized.
  * Per (block, k): one-hot S matrices (DVE: keytag == k*128+lr) turn the
    scatter-add into PE matmuls: A_bk^T = X_chunk^T @ S accumulated over
    the k-run's chunks, then out_b^T += W[k].T @ A_bk^T accumulated over k
    in PSUM.  Output lives transposed [ch, rows] so BN is a free-axis op.
  * BN stats accumulated per block from PSUM (fp32), all-reduced across
    cores; conv1 output normalized, block-transposed with one DMA-xbar op,
    all-gathered (fp16); conv2 epilogue fuses residual + relu over the
    whole shard.
"""

import os

import numpy as np


# ---------------------------------------------------------------- schedule
def _balance_blocks(deg, nblk):
    """Greedy: assign rows (desc total degree) to the block whose per-k load
    stays smallest; returns block id per row.  deg: [rows, K] int."""
    rows, K = deg.shape
    tot = deg.sum(1)
    order = np.argsort(-tot, kind="stable")
    load = np.zeros((nblk, K), np.float64)
    cnt = np.zeros(nblk, np.int64)
    assign = np.full(rows, -1, np.int64)
    for r in order:
        d = deg[r]
        proj = (load + d).max(1) + 0.02 * (load.sum(1) + tot[r]) / K
        proj[cnt >= 128] = 1e18
        b = int(np.argmin(proj))
        assign[r] = b
        load[b] += d
        cnt[b] += 1
    return assign


def _build_schedule(idx_in, idx_out, N, K, M, ncores, rpc, nblk):
    rpad = nblk * 128
    ki = np.repeat(np.arange(K, dtype=np.int64), M)
    ii = idx_in.reshape(-1).astype(np.int64)
    io = idx_out.reshape(-1).astype(np.int64)
    core = io // rpc
    loc = io - core * rpc

    # balanced row -> position (block*128 + rank) mapping per core
    newloc = np.zeros((ncores, rpc), np.int64)
    for c in range(ncores):
        deg = np.zeros((rpc, K), np.int32)
        m = core == c
        np.add.at(deg, (loc[m], ki[m]), 1)
        assign = _balance_blocks(deg, nblk)
        order = np.argsort(assign, kind="stable")
        rank = np.zeros(rpc, np.int64)
        start = 0
        for b in range(nblk):
            n = int((assign == b).sum())
            rank[order[start : start + n]] = np.arange(n)
            start += n
        newloc[c] = assign * 128 + rank

    pos = newloc[core, loc]  # position of each pair's output row
    blk = pos // 128
    lr = pos % 128
    # global position id of every input row (for gathers)
    gpos = (np.arange(N, dtype=np.int64) // rpc) * rpad + newloc[
        np.arange(N) // rpc, np.arange(N) % rpc
    ]

    ngroups = ncores * nblk * K
    gid = (core * nblk + blk) * K + ki
    counts = np.bincount(gid, minlength=ngroups).reshape(ncores, nblk, K)
    slots_bk = counts.max(axis=0)
    koff = np.zeros((nblk, K + 1), np.int64)
    np.cumsum(slots_bk, axis=1, out=koff[:, 1:])
    tot_b = koff[:, -1]
    Tb = np.maximum(1, -(-tot_b // 128))
    blk_chunk0 = np.zeros(nblk + 1, np.int64)
    np.cumsum(Tb, out=blk_chunk0[1:])
    Ttot = int(blk_chunk0[-1])

    order = np.argsort(gid, kind="stable")
    gid_s = gid[order]
    starts = np.zeros(ngroups + 1, np.int64)
    np.cumsum(counts.reshape(-1), out=starts[1:])
    rank = np.arange(len(gid_s), dtype=np.int64) - starts[gid_s]

    c_s = core[order]
    b_s = blk[order]
    k_s = ki[order]
    slot = blk_chunk0[b_s] * 128 + koff[b_s, k_s] + rank
    t_g = slot // 128
    p = slot % 128

    zrow = ncores * rpad  # zero row position
    gidx = np.full((ncores, 128, Ttot), zrow, np.int32)
    ktag = np.full((ncores, 128, Ttot), -1, np.int16)
    gidx[c_s, p, t_g] = gpos[ii[order]]
    ktag[c_s, p, t_g] = (k_s * 128 + lr[order]).astype(np.int16)
    return gidx, ktag, koff, Tb, blk_chunk0, Ttot, newloc


# ---------------------------------------------------------------- program
def _build_program(dims, koff, Tb, blk_chunk0, Ttot):
    import concourse.bacc as bacc
    import concourse.mybir as mybir
    import concourse.tile as tile
    from concourse.bass import IndirectOffsetOnAxis, _add_dep_helper

    N, C, K, ncores = dims["N"], dims["C"], dims["K"], dims["ncores"]
    nblk = dims["nblk"]
    rpad = nblk * 128
    npos = ncores * rpad  # global position space
    npad = npos + 128  # + zero rows
    eps = 1e-5
    pregather = dims.get("pregather", False)

    f16 = mybir.dt.float16
    f32 = mybir.dt.float32
    i32 = mybir.dt.int32
    Alu = mybir.AluOpType

    nc = bacc.Bacc(
        "TRN2", target_bir_lowering=False, debug=False, num_devices=ncores
    )

    xg = nc.dram_tensor("xg", [npad, C], f16, kind="ExternalInput")
    gi = nc.dram_tensor("gi", [128, Ttot], i32, kind="ExternalInput")
    kt = nc.dram_tensor("kt", [128, Ttot], mybir.dt.int16, kind="ExternalInput")
    wc = nc.dram_tensor("wc", [C, 2 * K * C], f16, kind="ExternalInput")
    gb = nc.dram_tensor("gb", [C, 4], f32, kind="ExternalInput")
    xs = nc.dram_tensor("xs", [rpad, C], f32, kind="ExternalInput")
    y = nc.dram_tensor("y", [rpad, C], f32, kind="ExternalOutput")
    if pregather:
        sl = nc.dram_tensor("sl", [Ttot * 128, C], f16, kind="ExternalInput")

    ag_in = nc.dram_tensor("ag_in", [rpad, C], f16, kind="Internal")
    ag_out = nc.dram_tensor(
        "ag_out", [npad, C], f16, kind="Internal", addr_space="Shared"
    )
    st_in = [
        nc.dram_tensor(f"st_in{i}", [C, 2], f32, kind="Internal") for i in (0, 1)
    ]
    st_out = [
        nc.dram_tensor(f"st_out{i}", [C, 2], f32, kind="Internal", addr_space="Shared")
        for i in (0, 1)
    ]
    rg = [list(range(ncores))]

    tmax = int(Tb.max())

    spans = []
    chunk_meta = []  # per block: per chunk (k0, span, off into S_all)
    sall_max = 0
    for b in range(nblk):
        row = []
        for k in range(K):
            s0, s1 = int(koff[b, k]), int(koff[b, k + 1])
            if s1 > s0:
                row.append((k, s0 // 128, (s1 - 1) // 128 + 1))
        spans.append(row)
        meta = []
        off = 0
        for tt in range(int(Tb[b])):
            ks = [k for (k, t0, t1) in row if t0 <= tt < t1]
            k0 = ks[0] if ks else 0
            sp = (ks[-1] - k0 + 1) if ks else 1
            meta.append((k0, sp, off))
            off += sp
        chunk_meta.append(meta)
        sall_max = max(sall_max, off)

    with tile.TileContext(nc) as tc:
        with (
            tc.tile_pool(name="const", bufs=1) as cpool,
            tc.tile_pool(name="big", bufs=1) as big,
            tc.tile_pool(name="gath", bufs=3) as gpool,
            tc.tile_pool(name="sel", bufs=2) as spool,
            tc.tile_pool(name="acp", bufs=4) as apool,
            tc.tile_pool(name="sq", bufs=4) as sqpool,
            tc.tile_pool(name="sc", bufs=1) as scpool,
            tc.tile_pool(name="psA", bufs=3, space="PSUM") as psA,
            tc.tile_pool(name="psO", bufs=2, space="PSUM") as psO,
        ):
            # ---------------- constants
            i16 = mybir.dt.int16
            iotak = cpool.tile([128, K * 128], i16)
            nc.gpsimd.iota(
                iotak[:], pattern=[[1, K * 128]], base=0, channel_multiplier=0
            )
            wsb = cpool.tile([C, 2 * K * C], f16)
            nc.sync.dma_start(wsb[:], wc[:])
            gis = cpool.tile([128, Ttot], i32)
            nc.sync.dma_start(gis[:], gi[:])
            kts = cpool.tile([128, Ttot], i16)
            nc.sync.dma_start(kts[:], kt[:])
            gbs = cpool.tile([C, 4], f32)
            nc.sync.dma_start(gbs[:], gb[:])
            ztile = cpool.tile([128, C], f16)
            nc.vector.memset(ztile[:], 0.0)
            ztail = nc.sync.dma_start(ag_out[npos:npad, :], ztile[: npad - npos, :])
            # residual shard, loaded early: xst[p, t, c] = xs[t*128+p, c]
            xst = big.tile([128, nblk * C], f32)
            nc.sync.dma_start(
                xst[:].rearrange("p (t c) -> p t c", c=C),
                xs[:, :].rearrange("(t p) c -> p t c", p=128),
            )

            outT = big.tile([C, rpad], f16)
            rowst = big.tile([C, rpad], f16)

            ag_inst = None
            for conv in range(2):
                src = xg if conv == 0 else ag_out
                rs1 = scpool.tile([C, 1], f32, tag=f"rs1_{conv}")
                rs2 = scpool.tile([C, 1], f32, tag=f"rs2_{conv}")
                nc.vector.memset(rs1[:], 0.0)
                nc.vector.memset(rs2[:], 0.0)

                # ---------------- block loop
                for b in range(nblk):
                    tb = int(Tb[b])
                    g0 = int(blk_chunk0[b])
                    gt = gpool.tile([128, tmax * C], f16, tag="gt")
                    if conv == 0 and pregather:
                        nc.sync.dma_start(
                            gt[:, : tb * C].rearrange("p (t c) -> p t c", c=C),
                            sl[g0 * 128 : (g0 + tb) * 128, :].rearrange(
                                "(t p) c -> p t c", p=128
                            ),
                        )
                    else:
                        for t in range(tb):
                            g = nc.gpsimd.indirect_dma_start(
                                out=gt[:, t * C : (t + 1) * C],
                                out_offset=None,
                                in_=src[:, :],
                                in_offset=IndirectOffsetOnAxis(
                                    ap=gis[:, g0 + t : g0 + t + 1], axis=0
                                ),
                            )
                            if conv == 1:
                                _add_dep_helper(g.ins, ag_inst.ins, True, "wait ag")
                                _add_dep_helper(g.ins, ztail.ins, True, "wait zt")

                    meta = chunk_meta[b]
                    S_all = spool.tile([128, sall_max * 128], f16, tag="S")
                    for tt in range(tb):
                        k0, sp, off = meta[tt]
                        nc.vector.tensor_tensor(
                            out=S_all[:, off * 128 : (off + sp) * 128],
                            in0=kts[:, g0 + tt : g0 + tt + 1].to_broadcast(
                                [128, sp * 128]
                            ),
                            in1=iotak[:, k0 * 128 : (k0 + sp) * 128],
                            op=Alu.is_equal,
                        )

                    ob = psO.tile([C, 128], f32, tag="ob")
                    row = spans[b]
                    nk = len(row)
                    for ik, (k, t0, t1) in enumerate(row):
                        apt = psA.tile([128, 128], f32, tag="A")
                        for tt in range(t0, t1):
                            k0, sp, off = meta[tt]
                            sidx = off + (k - k0)
                            nc.tensor.matmul(
                                out=apt[:],
                                lhsT=gt[:, tt * C : (tt + 1) * C],
                                rhs=S_all[:, sidx * 128 : (sidx + 1) * 128],
                                start=(tt == t0),
                                stop=(tt == t1 - 1),
                            )
                        a_s = apool.tile([128, 128], f16, tag="a")
                        if ik % 2 == 0:
                            nc.vector.tensor_copy(a_s[:], apt[:])
                        else:
                            nc.scalar.copy(a_s[:], apt[:])
                        nc.tensor.matmul(
                            out=ob[:],
                            lhsT=wsb[:, (conv * K + k) * C : (conv * K + k + 1) * C],
                            rhs=a_s[:],
                            start=(ik == 0),
                            stop=(ik == nk - 1),
                        )
                    nc.scalar.copy(outT[:, b * 128 : (b + 1) * 128], ob[:])
                    # per-block BN stats (sum from PSUM fp32; sumsq from fp16 copy)
                    ots = outT[:, b * 128 : (b + 1) * 128]
                    t1_ = sqpool.tile([C, 1], f32, tag="t1")
                    nc.vector.tensor_reduce(
                        out=t1_[:], in_=ob[:], axis=mybir.AxisListType.X, op=Alu.add
                    )
                    sq = sqpool.tile([C, 128], f32, tag="sqf")
                    nc.vector.tensor_tensor(out=sq[:], in0=ots, in1=ots, op=Alu.mult)
                    t2_ = sqpool.tile([C, 1], f32, tag="t2")
                    nc.vector.tensor_reduce(
                        out=t2_[:], in_=sq[:], axis=mybir.AxisListType.X, op=Alu.add
                    )
                    nc.vector.tensor_tensor(rs1[:], rs1[:], t1_[:], op=Alu.add)
                    nc.vector.tensor_tensor(rs2[:], rs2[:], t2_[:], op=Alu.add)

                # ---------------- BN: allreduce stats, scale/shift
                stg = scpool.tile([C, 2], f32, tag=f"stg_{conv}")
                nc.vector.tensor_copy(stg[:, 0:1], rs1[:])
                nc.vector.tensor_copy(stg[:, 1:2], rs2[:])
                d_st = nc.sync.dma_start(st_in[conv][:, :], stg[:])
                cc_st = nc.gpsimd.collective_compute(
                    "AllReduce",
                    Alu.add,
                    replica_groups=rg,
                    ins=[st_in[conv][:, :]],
                    outs=[st_out[conv][:, :]],
                )
                _add_dep_helper(cc_st.ins, d_st.ins, True, "stats in")
                stg2 = scpool.tile([C, 2], f32, tag=f"stg2_{conv}")
                d_st2 = nc.sync.dma_start(stg2[:], st_out[conv][:, :])
                _add_dep_helper(d_st2.ins, cc_st.ins, True, "stats out")

                mean = scpool.tile([C, 1], f32, tag=f"mean_{conv}")
                nc.vector.tensor_scalar(
                    out=mean[:], in0=stg2[:, 0:1], scalar1=1.0 / N, scalar2=None,
                    op0=Alu.mult,
                )
                var = scpool.tile([C, 1], f32, tag=f"var_{conv}")
                nc.vector.scalar_tensor_tensor(
                    out=var[:], in0=mean[:], scalar=-1.0, in1=mean[:],
                    op0=Alu.mult, op1=Alu.mult,
                )
                nc.vector.scalar_tensor_tensor(
                    out=var[:], in0=stg2[:, 1:2], scalar=1.0 / N, in1=var[:],
                    op0=Alu.mult, op1=Alu.add,
                )
                nc.vector.tensor_scalar_add(var[:], var[:], eps)
                sd = scpool.tile([C, 1], f32, tag=f"sd_{conv}")
                nc.scalar.sqrt(sd[:], var[:])
                rstd = scpool.tile([C, 1], f32, tag=f"rstd_{conv}")
                nc.vector.reciprocal(rstd[:], sd[:])
                scale = scpool.tile([C, 1], f32, tag=f"scale_{conv}")
                nc.vector.tensor_tensor(
                    out=scale[:], in0=gbs[:, 2 * conv : 2 * conv + 1], in1=rstd[:],
                    op=Alu.mult,
                )
                shift = scpool.tile([C, 1], f32, tag=f"shift_{conv}")
                nc.vector.scalar_tensor_tensor(
                    out=shift[:], in0=mean[:], scalar=-1.0, in1=scale[:],
                    op0=Alu.mult, op1=Alu.mult,
                )
                nc.vector.tensor_tensor(
                    out=shift[:], in0=shift[:],
                    in1=gbs[:, 2 * conv + 1 : 2 * conv + 2], op=Alu.add,
                )
                # normalize in place (column-major, per-partition scalars)
                nc.vector.tensor_scalar(
                    out=outT[:], in0=outT[:], scalar1=scale[:], scalar2=shift[:],
                    op0=Alu.mult, op1=Alu.add,
                )
                if conv == 0:
                    nc.vector.tensor_scalar_max(outT[:], outT[:], 0.0)  # relu
                    # one-shot block transpose: rowst[p, t, c] = outT[c, t*128+p]
                    nc.sync.dma_start_transpose(
                        rowst[:].rearrange("p (t c) -> p t c", c=C), outT[:]
                    )
                    d1 = nc.sync.dma_start(
                        ag_in[:, :].rearrange("(t p) c -> p t c", p=128),
                        rowst[:].rearrange("p (t c) -> p t c", c=C),
                    )
                    ag_inst = nc.gpsimd.collective_compute(
                        "AllGather",
                        Alu.bypass,
                        replica_groups=rg,
                        ins=[ag_in[:, :]],
                        outs=[ag_out[0:npos, :]],
                    )
                    _add_dep_helper(ag_inst.ins, d1.ins, True, "ag in ready")
                else:
                    nc.sync.dma_start_transpose(
                        rowst[:].rearrange("p (t c) -> p t c", c=C), outT[:]
                    )
                    # y = relu(out2 + x), fused over the whole shard
                    nc.vector.tensor_tensor(
                        out=xst[:], in0=rowst[:], in1=xst[:], op=Alu.add
                    )
                    nc.vector.tensor_scalar_max(xst[:], xst[:], 0.0)
                    nc.sync.dma_start(
                        y[:, :].rearrange("(t p) c -> p t c", p=128),
                        xst[:].rearrange("p (t c) -> p t c", c=C),
                    )

    nc.compile()
    return nc


# ---------------------------------------------------------------- runner
def _prepare_inputs(x, w1, gamma1, beta1, w2, gamma2, beta2, gidx, ktag, newloc, dims):
    N, C, K, ncores = dims["N"], dims["C"], dims["K"], dims["ncores"]
    rpc, nblk = dims["rpc"], dims["nblk"]
    rpad = nblk * 128
    npos = ncores * rpad
    npad = npos + 128

    x = np.asarray(x, np.float32)
    # permute into position space
    xg = np.zeros((npad, C), np.float16)
    xpos = np.zeros((npos, C), np.float32)
    for c in range(ncores):
        xpos[c * rpad + newloc[c]] = x[c * rpc : (c + 1) * rpc]
    xg[:npos] = xpos.astype(np.float16)

    wcat = np.concatenate(
        [
            np.transpose(np.asarray(w1, np.float16), (1, 0, 2)).reshape(C, K * C),
            np.transpose(np.asarray(w2, np.float16), (1, 0, 2)).reshape(C, K * C),
        ],
        axis=1,
    )
    gbcat = np.stack(
        [
            np.asarray(gamma1, np.float32),
            np.asarray(beta1, np.float32),
            np.asarray(gamma2, np.float32),
            np.asarray(beta2, np.float32),
        ],
        axis=1,
    )
    in_maps = []
    for c in range(ncores):
        m = {
            "xg": xg,
            "gi": np.ascontiguousarray(gidx[c]),
            "kt": np.ascontiguousarray(ktag[c]),
            "wc": wcat,
            "gb": gbcat,
            "xs": np.ascontiguousarray(xpos[c * rpad : (c + 1) * rpad]),
        }
        if dims.get("pregather", False):
            g = gidx[c]  # [128, Ttot]
            flat = g.T.reshape(-1)
            m["sl"] = np.ascontiguousarray(xg[flat])
        in_maps.append(m)
    return in_maps


def _dims(N, C, K, M, ncores=8):
    rpc = N // ncores
    nblk = -(-rpc // 128)
    return dict(
        N=N, C=C, K=K, M=M, ncores=ncores, rpc=rpc, nblk=nblk,
        pregather=os.environ.get("PREGATHER", "0") == "1",
    )


def assemble_output(results, dims, newloc):
    ncores, rpc, nblk = dims["ncores"], dims["rpc"], dims["nblk"]
    rpad = nblk * 128
    y = np.empty((ncores * rpc, dims["C"]), np.float32)
    for c in range(ncores):
        yc = np.asarray(results[c]["y"], np.float32)
        y[c * rpc : (c + 1) * rpc] = yc[newloc[c]]
    return y


def build_all(x, w1, gamma1, beta1, w2, gamma2, beta2, idx_in, idx_out, ncores=8):
    K, M = idx_in.shape
    N, C = x.shape
    dims = _dims(N, C, K, M, ncores)
    gidx, ktag, koff, Tb, blk_chunk0, Ttot, newloc = _build_schedule(
        np.asarray(idx_in), np.asarray(idx_out), N, K, M, ncores, dims["rpc"],
        dims["nblk"],
    )
    nc = _build_program(dims, koff, Tb, blk_chunk0, Ttot)
    in_maps = _prepare_inputs(
        np.asarray(x), w1, gamma1, beta1, w2, gamma2, beta2, gidx, ktag, newloc, dims
    )
    return nc, in_maps, dims, newloc


def kernel(x, w1, gamma1, beta1, w2, gamma2, beta2, idx_in, idx_out):
    from concourse.bass_utils import run_bass_kernel_spmd

    nc, in_maps, dims, newloc = build_all(
        x, w1, gamma1, beta1, w2, gamma2, beta2, idx_in, idx_out
    )
    ncores = dims["ncores"]
    res = run_bass_kernel_spmd(nc, in_maps, core_ids=list(range(ncores)))
    return np.ascontiguousarray(assemble_output(res.results, dims, newloc))

